# revision 1
# baseline (speedup 1.0000x reference)
"""Trainium2 Bass kernel for nn_CausalSelfAttention_28467043237962.

Sharding: 8 cores = 2 batches x 4 head-groups (4 heads / 256 dims each).
Per core: fused QKV projection (bf16 matmuls, fp32 PSUM), per-head RMS-norm
(squares + norm-multiplies on gpsimd, segmented reduce on DVE) + RoPE (16
active frequencies, DVE), causal attention in transposed-scores layout
(scores_T[s,t]; softmax denominators come from a [v|ones64] column block in
the AV matmul and are divided out with a PSUM/SBUF cross-partition-base
multiply -- no DRAM roundtrip), output projection, host-side sum of the 4
per-batch bf16 partials. All host-side inputs are pre-arranged into
partition-major layouts so every DMA is a contiguous per-partition stripe.
Mask blocks are classified on the host (skip/pass/tri/general); scores
matmuls and exp are trimmed at the causal diagonal.

Self-contained: builds one SPMD Bass program and runs it on cores 0-7 via
concourse.bass_utils.run_bass_kernel_spmd.
"""
import sys

sys.path.insert(0, "/opt/trn_rl_repo")

from contextlib import ExitStack

import numpy as np
import ml_dtypes

import concourse.bass as bass
import concourse.tile as tile
import concourse.mybir as mybir
from concourse.vector_clock import ScopedClock
from concourse.bass_utils import run_bass_kernel_spmd

F32 = mybir.dt.float32
BF16 = mybir.dt.bfloat16

B, T, DIM = 2, 2048, 1024
H, HD = 16, 64
SCALE = 0.12
ROPE_BASE = 1024.0
EPS = 1e-6
G = 4          # head-groups = cores per batch
HPG = 4        # heads per group
GD = HPG * HD  # 256 dims per group
NT = T // 128  # 16 s/t tiles of 128
NJ = T // 512  # 4 t-blocks of 512

TRACE = False          # set by test.py for profiling runs
DBG = False            # add debug dumps of intermediates
LAST_EXEC_NS = None    # filled when TRACE


class _TileContextFixed(tile.TileContext):
    """Workaround for this container's walrus build: the kernel-tail InstDrain
    may carry only one sync wait. Spread the tail waits over single-wait NOPs
    on the sync engine before a wait-free drain."""

    def _drain_and_barrier(self, tick_clock, wait_clock):
        nc = self.nc
        collector = nc.sync.nop(nofuse=True, hint="tail_wait_collector")
        wait_clock.add_sem_waits(
            collector.ins, ScopedClock({None: tick_clock.global_clock})
        )
        si = collector.ins.sync_info
        waits = list(si.on_wait or [])
        if len(waits) > 1:
            si.on_wait = waits[:1]
            for w in waits[1:]:
                extra = nc.sync.nop(nofuse=True, hint="tail_wait")
                esi = extra.ins.sync_info
                if esi is None:
                    extra.ins.sync_info = mybir.SyncInfo(on_wait=[w], on_update=[])
                else:
                    esi.on_wait = [w]
        nc.sync.drain()
        nc.all_engine_barrier()
        assert self.sems is not None
        popped = nc._tile_sem_poison_stack.pop()
        assert popped is self._sem_poison
        nc.clear_and_free_semaphores(list(self.sems.allocated().values()))
        nc.all_engine_barrier()


def _split_excess_waits(nc, max_waits=1):
    """This container's walrus build rejects instructions carrying more than
    one embedded sync wait. Move excess waits onto dedicated NOPs inserted
    just before the instruction on the same engine."""
    ctr = [0]
    for func in nc.m.functions:
        for block in func.blocks:
            out = []
            for inst in block.instructions:
                si = inst.sync_info
                waits = list(si.on_wait) if si and si.on_wait else []
                limit = 0 if isinstance(inst, mybir.InstDrain) else max_waits
                if len(waits) > limit:
                    keep = waits[:limit]
                    extra = waits[limit:]
                    for w in extra:
                        ctr[0] += 1
                        nop = mybir.InstNoOp(
                            name=f"waitnop-{ctr[0]}",
                            sync_info=mybir.SyncInfo(on_wait=[w], on_update=[]),
                            bass_nofuse=True,
                            engine=inst.engine,
                        )
                        out.append(nop)
                    si.on_wait = keep
                out.append(inst)
            block.instructions = out


def _rope_tables():
    keep = HD // 4  # 16 active frequencies; dims 16:32 of each half are identity
    active = (1.0 / ROPE_BASE) ** np.linspace(0.0, 1.0, keep, dtype=np.float32)
    th = np.arange(T, dtype=np.float32)[:, None] * active[None, :]
    return np.cos(th).astype(np.float32), np.sin(th).astype(np.float32)


def _classify_blocks(mask):
    """mask [T,T] additive, indexed (t, s). Block = (s-tile i of 128) x
    (t-block jb of 512). Returns cls[i][jb] in {skip, pass, tri, gen}."""
    cls = []
    for i in range(NT):
        row = []
        for jb in range(NJ):
            blk = mask[512 * jb:512 * (jb + 1), 128 * i:128 * (i + 1)]
            big_neg = blk <= -1e8
            zero = blk == 0.0
            if big_neg.all():
                row.append("skip")
            elif zero.all():
                row.append("pass")
            elif (big_neg | zero).all():
                tt = np.arange(512 * jb, 512 * (jb + 1))[:, None]
                ss = np.arange(128 * i, 128 * (i + 1))[None, :]
                row.append("tri" if np.array_equal(zero, tt >= ss) else "gen")
            else:
                row.append("gen")
        cls.append(row)
    return cls


def _build_program(cls, has_gen):
    nc = bass.Bass()
    xP = nc.declare_dram_parameter("xP", [128, 8, T], BF16, isOutput=False)
    wqP = nc.declare_dram_parameter("wqP", [128, 8, 3 * GD], BF16, isOutput=False)
    woP = nc.declare_dram_parameter("woP", [128, 2, DIM], BF16, isOutput=False)
    veP = nc.declare_dram_parameter("veP", [128, NT, GD], BF16, isOutput=False)
    lamP = nc.declare_dram_parameter("lamP", [128, 1], F32, isOutput=False)
    rcP = nc.declare_dram_parameter("rcP", [128, NT, 16], BF16, isOutput=False)
    rsP = nc.declare_dram_parameter("rsP", [128, NT, 16], BF16, isOutput=False)
    triP = nc.declare_dram_parameter("triP", [128, 4, 512], BF16, isOutput=False)
    md = None
    if has_gen:
        md = nc.declare_dram_parameter("maskdiv", [T, T], F32, isOutput=False)
    outP = nc.declare_dram_parameter("outP", [DIM, T], BF16, isOutput=True)
    if DBG:
        dbg_qkT = nc.declare_dram_parameter("dbg_qkT", [128, 4, T], BF16,
                                            isOutput=True)
        dbg_yT = nc.declare_dram_parameter("dbg_yT", [128, 2, T], BF16,
                                           isOutput=True)
        dbg_va = nc.declare_dram_parameter("dbg_va", [128, NT, HPG, 128],
                                           BF16, isOutput=True)
        dbg_scl = nc.declare_dram_parameter("dbg_scl", [128, NT, HPG], F32,
                                            isOutput=True)
        dbg_den = nc.declare_dram_parameter("dbg_den", [8, 1024], F32,
                                            isOutput=True)
        dbg_av = nc.declare_dram_parameter("dbg_av", [128, 1024], F32,
                                           isOutput=True)
    rec_dram = nc.dram_tensor("rec_scratch", [HPG, T], F32)

    # per-jb: first/last valid s-tile for AV accumulation start/stop
    first_i = [None] * NJ
    last_i = [None] * NJ
    for jb in range(NJ):
        valid = [i for i in range(NT) if cls[i][jb] != "skip"]
        if valid:
            first_i[jb] = valid[0]
            last_i[jb] = valid[-1]

    with _TileContextFixed(nc) as tc, ExitStack() as ctx:
        S = ctx.enter_context(tc.tile_pool(name="singles", bufs=1))

        # ---- input DMAs: interleave per-ds (wq chunk, x chunk) pairs across
        # the two hwdge queues so the first QKV matmuls can start early.
        wq_sb = S.tile([128, 8, 3 * GD], BF16, tag="wq_sb")
        x_sb = S.tile([128, 8, T], BF16, tag="x_sb")
        for ds in range(8):
            eng = nc.sync if ds % 2 == 0 else nc.scalar
            eng.dma_start(out=wq_sb[:, ds, :], in_=wqP[:, ds, :])
            eng.dma_start(out=x_sb[:, ds, :], in_=xP[:, ds, :])
        ve_sb = S.tile([128, NT, GD], BF16, tag="ve_sb")
        nc.scalar.dma_start(out=ve_sb, in_=veP[:, :, :])
        lam_sb = S.tile([128, 1], F32, tag="lam_sb")
        nc.sync.dma_start(out=lam_sb, in_=lamP[:, :])
        rc_sb = S.tile([128, NT, 16], BF16, tag="rc_sb")
        nc.sync.dma_start(out=rc_sb, in_=rcP[:, :, :])
        rs_sb = S.tile([128, NT, 16], BF16, tag="rs_sb")
        nc.sync.dma_start(out=rs_sb, in_=rsP[:, :, :])
        tri_sb = S.tile([128, 4, 512], BF16, tag="tri_sb")
        nc.sync.dma_start(out=tri_sb, in_=triP[:, :, :])
        wo_sb = S.tile([128, 2, DIM], BF16, tag="wo_sb")
        nc.scalar.dma_start(out=wo_sb, in_=woP[:, :, :])

        eps_sb = S.tile([128, 1], F32, tag="eps_sb")
        nc.gpsimd.memset(eps_sb, EPS)
        ident = S.tile([128, 128], BF16, tag="ident")
        from concourse.masks import make_identity
        make_identity(nc, ident)

        # v_aug[p, tt, h, 0:128]: even h -> [v | ones], odd h -> [ones | v]
        v_aug = S.tile([128, NT, HPG, 128], BF16, tag="v_aug")
        v5 = v_aug.rearrange("p t (a b) c -> p t a b c", b=2)
        nc.gpsimd.memset(v5[:, :, :, 0, 64:128], 1.0)
        nc.gpsimd.memset(v5[:, :, :, 1, 0:64], 1.0)

        qkv_all = S.tile([128, NT, 3 * GD], BF16, tag="qkv_all")
        # qkT[:, 0:2, :] = qT pairs, [:, 2:4, :] = kT pairs
        qkT = S.tile([128, 4, T], BF16, tag="qkT")
        yT = S.tile([128, 2, T], BF16, tag="yT")
        scl_sb = S.tile([128, NT, HPG], F32, tag="scl_sb")

        # One shared PSUM pool: tags "sc" and "av" (2 bufs x 4KB each =
        # 8 banks). Stage A borrows "sc" slots for QKV psum and "av" slots
        # for transpose psum, so stages A and B/C can interleave.
        PS = ctx.enter_context(tc.tile_pool(name="ps", bufs=2, space="PSUM"))
        A = ctx.enter_context(tc.tile_pool(name="a_sb", bufs=2))
        EX = ctx.enter_context(tc.tile_pool(name="ex_sb", bufs=3))
        # The exp writes only [lo:hi] of each ex tile; the tri multiply
        # zeroes the stale remainder, which is only safe if it is finite
        # (0*NaN=NaN). Scrub the three pool slots once so stale content is
        # always a previous (finite) exp output.
        for _k in range(3):
            ex0 = EX.tile([128, 1024], BF16, tag="ex", name="ex_scrub")
            nc.gpsimd.memset(ex0, 0.0)
        NR = ctx.enter_context(tc.tile_pool(name="rec_sb", bufs=2))
        OS = ctx.enter_context(tc.tile_pool(name="os_sb", bufs=4))
        MD = ctx.enter_context(tc.tile_pool(name="md_sb", bufs=2))

        def emit_a(quarter):
            """QKV + norm + rope + blend + transpose for t-tiles
            [4*quarter, 4*quarter+4)."""
            NQ = 4
            tts = range(NQ * quarter, NQ * quarter + NQ)
            hsl = slice(NQ * quarter, NQ * quarter + NQ)
            sq = A.tile([128, NQ, 3 * GD], BF16, tag="sq", name="sq")
            ms = A.tile([128, NQ, 12], F32, tag="ms", name="ms")
            for tt in tts:
                ts = slice(128 * tt, 128 * (tt + 1))
                qp = PS.tile([128, 1024], F32, tag="sc", name="qp")
                for ds in range(8):
                    lhsT = x_sb[:, ds, ts]
                    nc.tensor.matmul(
                        qp[:, 0:512], lhsT, wq_sb[:, ds, 0:512],
                        start=(ds == 0), stop=(ds == 7))
                    nc.tensor.matmul(
                        qp[:, 512:768], lhsT, wq_sb[:, ds, 512:768],
                        start=(ds == 0), stop=(ds == 7))
                nc.scalar.copy(qkv_all[:, tt, :], qp[:, 0:768])
                nc.scalar.activation(
                    sq[:, tt - NQ * quarter, :], qp[:, 0:768],
                    mybir.ActivationFunctionType.Square)
                nc.vector.tensor_reduce(
                    ms[:, tt - NQ * quarter, :],
                    sq[:, tt - NQ * quarter, :].rearrange(
                        "p (g d) -> p g d", d=HD),
                    axis=mybir.AxisListType.X, op=mybir.AluOpType.add)

            srt = A.tile([128, NQ, 12], F32, tag="srt", name="srt")
            nc.scalar.activation(
                srt, ms, mybir.ActivationFunctionType.Sqrt,
                bias=eps_sb[:, 0:1], scale=1.0 / HD)
            rstd = A.tile([128, NQ, 12], F32, tag="rstd", name="rstd")
            nc.vector.reciprocal(rstd, srt)
            # k-norm rides the exp() scale in stage B (SCALE * rstd_k)
            nc.vector.tensor_scalar_mul(
                scl_sb[:, hsl, :], rstd[:, :, 4:8], SCALE)

            qk4 = qkv_all[:, hsl, :].rearrange("p t (g d) -> p t g d", d=HD)
            if has_gen:
                # general-mask fallback: normalize k on-device too
                nc.vector.tensor_tensor(
                    qk4[:, :, 4:8, :], qk4[:, :, 4:8, :],
                    rstd[:, :, 4:8, None].to_broadcast([128, NQ, 4, HD]),
                    mybir.AluOpType.mult)
            nc.vector.tensor_tensor(
                qk4[:, :, 0:4, :], qk4[:, :, 0:4, :],
                rstd[:, :, 0:4, None].to_broadcast([128, NQ, 4, HD]),
                mybir.AluOpType.mult)
            nc.vector.tensor_tensor(
                qk4[:, :, 8:12, :], qk4[:, :, 8:12, :],
                rstd[:, :, 8:12, None].to_broadcast([128, NQ, 4, HD]),
                mybir.AluOpType.mult)

            # rope on q,k: active dims 0:16 and 32:48 per head
            v6 = qkv_all[:, hsl, 0:512].rearrange(
                "p t (sg d) -> p t sg d", d=HD)
            x0 = v6[:, :, :, 0:16]
            x32 = v6[:, :, :, 32:48]
            cb = rc_sb[:, hsl, None, :].to_broadcast([128, NQ, 8, 16])
            sb = rs_sb[:, hsl, None, :].to_broadcast([128, NQ, 8, 16])
            ra = A.tile([128, NQ, 8, 16], BF16, tag="ra", name="ra")
            rb = A.tile([128, NQ, 8, 16], BF16, tag="rb", name="rb")
            nc.vector.tensor_tensor(ra, x0, sb, mybir.AluOpType.mult)
            nc.vector.tensor_tensor(rb, x32, sb, mybir.AluOpType.mult)
            nc.vector.tensor_tensor(x0, x0, cb, mybir.AluOpType.mult)
            nc.vector.tensor_tensor(x32, x32, cb, mybir.AluOpType.mult)
            nc.vector.tensor_tensor(x0, x0, rb, mybir.AluOpType.add)
            nc.vector.tensor_tensor(x32, x32, ra, mybir.AluOpType.subtract)

            # v blend: v_aug = lambda0 * v_n + ve' (ve is lambda1-scaled)
            vv = qkv_all[:, hsl, 512:768].rearrange(
                "p t (a b d) -> p t a b d", a=2, d=HD)
            vev = ve_sb[:, hsl, :].rearrange(
                "p t (a b d) -> p t a b d", a=2, d=HD)
            for a in range(2):
                nc.vector.scalar_tensor_tensor(
                    out=v5[:, hsl, a, 0, 0:64], in0=vv[:, :, a, 0, :],
                    scalar=lam_sb[:, 0:1], in1=vev[:, :, a, 0, :],
                    op0=mybir.AluOpType.mult, op1=mybir.AluOpType.add)
                nc.vector.scalar_tensor_tensor(
                    out=v5[:, hsl, a, 1, 64:128], in0=vv[:, :, a, 1, :],
                    scalar=lam_sb[:, 0:1], in1=vev[:, :, a, 1, :],
                    op0=mybir.AluOpType.mult, op1=mybir.AluOpType.add)

            # transpose q,k head-pairs to [hd, t]
            for tt in tts:
                ts = slice(128 * tt, 128 * (tt + 1))
                tp_t = PS.tile([128, 2048], BF16, tag="av", name="tp_t")
                tp = tp_t[:, 0:512]
                for ec in range(4):
                    nc.tensor.transpose(
                        tp[:, 128 * ec:128 * (ec + 1)],
                        qkv_all[:, tt, 128 * ec:128 * (ec + 1)], ident)
                nc.vector.tensor_scalar_mul(
                    qkT[:, :, ts],
                    tp.rearrange("p (e c) -> p e c", c=128), 1.0)

        def emit_b(jj):
            """Attention for t-columns [1024*jj, 1024*(jj+1))."""
            for h in range(HPG):
                par, pair = h % 2, h // 2
                rlo, rhi = 64 * par, 64 * par + 64
                dlo = 64 - rlo          # den rows (ones-cols of v_aug)
                av = PS.tile([128, 1024], F32, tag="av", name="av")
                for i in range(NT):
                    jbs = [jb for jb in (2 * jj, 2 * jj + 1)
                           if cls[i][jb] != "skip"]
                    if not jbs:
                        continue
                    sc = PS.tile([128, 1024], F32, tag="sc", name="sc")
                    trims = {}
                    for jb in jbs:
                        off = 512 * (jb - 2 * jj)
                        trim = (128 * (i - 4 * jb)
                                if cls[i][jb] == "tri" else 0)
                        trims[jb] = trim
                        nc.tensor.matmul(
                            sc[:, off + trim:off + 512],
                            qkT[rlo:rhi, 2 + pair, 128 * i:128 * (i + 1)],
                            qkT[rlo:rhi, pair,
                                512 * jb + trim:512 * (jb + 1)],
                            start=True, stop=True)
                        if cls[i][jb] == "gen":
                            mdt = MD.tile([128, 512], F32, tag="mdt",
                                          name="mdt")
                            nc.sync.dma_start(
                                out=mdt,
                                in_=md[512 * jb:512 * (jb + 1),
                                       128 * i:128 * (i + 1)].rearrange(
                                           "t s -> s t"))
                            nc.vector.tensor_tensor(
                                sc[:, off:off + 512], sc[:, off:off + 512],
                                mdt, mybir.AluOpType.add)
                    ex = EX.tile([128, 1024], BF16, tag="ex", name="ex")
                    lo = 512 * (min(jbs) - 2 * jj) + trims[min(jbs)]
                    hi = 512 * (max(jbs) - 2 * jj) + 512
                    escale = (SCALE if has_gen else scl_sb[:, i, h:h + 1])
                    nc.scalar.activation(
                        ex[:, lo:hi], sc[:, lo:hi],
                        mybir.ActivationFunctionType.Exp, scale=escale)
                    for jb in jbs:
                        off = 512 * (jb - 2 * jj)
                        if cls[i][jb] == "tri":
                            nc.vector.tensor_tensor(
                                ex[:, off:off + 512],
                                ex[:, off:off + 512],
                                tri_sb[:, i % 4, :],
                                mybir.AluOpType.mult)
                        nc.tensor.matmul(
                            av[:, off:off + 512],
                            v_aug[:, i, h, :],
                            ex[:, off:off + 512],
                            start=(first_i[jb] == i),
                            stop=(last_i[jb] == i))
                # empty column-blocks (pathological masks): den=1, y=0
                for jb in (2 * jj, 2 * jj + 1):
                    if first_i[jb] is None:
                        off = 512 * (jb - 2 * jj)
                        nc.vector.memset(av[:, off:off + 512], 1.0)
                # normalize: y rows at rlo:rhi, replicated den at dlo:dlo+64.
                # Pull one den row to SBUF, scatter it to [128,8] so the
                # reciprocal is cheap, then broadcast the result back over
                # 64 partitions via a stride-0 DRAM read.
                den_sb = NR.tile([1, 1024], F32, tag="den_sb", name="den_sb")
                nc.vector.tensor_scalar_mul(den_sb, av[dlo:dlo + 1, :], 1.0)
                den_pk = NR.tile([128, 8], F32, tag="den_pk", name="den_pk")
                nc.sync.dma_start(out=den_pk, in_=den_sb)
                if DBG:
                    nc.sync.dma_start(out=dbg_den[4 * jj + h, :], in_=den_sb)
                    if h == 0 and jj == 0:
                        av_dump = NR.tile([128, 1024], F32, tag="av_dump",
                                          name="av_dump")
                        nc.scalar.copy(av_dump, av)
                        nc.sync.dma_start(out=dbg_av[:, :], in_=av_dump)
                rec_pk = NR.tile([128, 8], F32, tag="rec_pk", name="rec_pk")
                nc.vector.reciprocal(rec_pk, den_pk)
                hsl2 = slice(1024 * jj, 1024 * (jj + 1))
                nc.sync.dma_start(out=rec_dram[h, hsl2], in_=rec_pk)
                rrow_ap = rec_dram[h, hsl2]
                rec_bc = bass.AP(
                    tensor=rrow_ap.tensor,
                    offset=rrow_ap.offset,
                    ap=[[0, 64]] + [list(p) for p in rrow_ap.ap])
                rec = NR.tile([64, 1024], F32, tag="rec", name="rec")
                nc.sync.dma_start(out=rec, in_=rec_bc)
                nc.vector.tensor_tensor(
                    yT[rlo:rhi, pair, 1024 * jj:1024 * (jj + 1)],
                    av[rlo:rhi, :], rec,
                    mybir.AluOpType.mult)

        def emit_c(jj):
            """Output projection for t-columns [1024*jj, 1024*(jj+1))."""
            for ec in range(8):
                for tb in (2 * jj, 2 * jj + 1):
                    po_t = PS.tile([128, 1024], F32, tag="sc", name="po_t")
                    po = po_t[:, 0:512]
                    for dc in range(2):
                        nc.tensor.matmul(
                            po,
                            wo_sb[:, dc, 128 * ec:128 * (ec + 1)],
                            yT[:, dc, 512 * tb:512 * (tb + 1)],
                            start=(dc == 0), stop=(dc == 1))
                    os = OS.tile([128, 512], BF16, tag="os", name="os")
                    if (ec + tb) % 4 == 0:
                        nc.scalar.copy(os, po)
                    else:
                        nc.vector.tensor_copy(out=os, in_=po)
                    nc.sync.dma_start(
                        out=outP[128 * ec:128 * (ec + 1),
                                 512 * tb:512 * (tb + 1)],
                        in_=os)

        emit_a(0)
        emit_a(1)
        emit_a(2)
        emit_a(3)
        emit_b(0)
        emit_c(0)
        emit_b(1)
        emit_c(1)
        if DBG:
            nc.sync.dma_start(out=dbg_qkT[:, :, :], in_=qkT)
            nc.sync.dma_start(out=dbg_yT[:, :, :], in_=yT)
            nc.sync.dma_start(out=dbg_va[:, :, :, :], in_=v_aug)
            nc.sync.dma_start(out=dbg_scl[:, :, :], in_=scl_sb)
    _split_excess_waits(nc)
    return nc


def kernel(x, ve, sa_lambdas, attn_mask, qkvo_w):
    global LAST_EXEC_NS
    x = np.ascontiguousarray(np.asarray(x, np.float32))
    ve = np.ascontiguousarray(np.asarray(ve, np.float32))
    sa_lambdas = np.asarray(sa_lambdas, np.float32)
    attn_mask = np.asarray(attn_mask, np.float32)
    qkvo_w = np.asarray(qkvo_w, np.float32)

    ropeC, ropeS = _rope_tables()
    mask = attn_mask[0, 0]
    cls = _classify_blocks(mask)
    has_gen = any(c == "gen" for row in cls for c in row)

    nc = _build_program(cls, has_gen)

    iota = np.arange(512)
    part = np.arange(128)
    diff = part[:, None] - iota[None, :]                               # p - c
    tri01 = np.stack([(diff <= -128 * m) for m in range(4)], 0)
    tri01 = np.ascontiguousarray(
        tri01.transpose(1, 0, 2)).astype(ml_dtypes.bfloat16)           # [128,4,512]
    lam0 = np.full((128, 1), sa_lambdas[0], np.float32)
    rcP = np.ascontiguousarray(
        ropeC.reshape(NT, 128, 16).transpose(1, 0, 2)).astype(ml_dtypes.bfloat16)
    rsP = np.ascontiguousarray(
        ropeS.reshape(NT, 128, 16).transpose(1, 0, 2)).astype(ml_dtypes.bfloat16)
    maskdiv = (mask / SCALE).astype(np.float32) if has_gen else None

    in_maps = []
    for c in range(8):
        b, g = c // G, c % G
        sl = slice(GD * g, GD * (g + 1))
        wqkvT = np.concatenate([qkvo_w[k][sl, :] for k in range(3)], 0).T
        m = {
            "xP": np.ascontiguousarray(
                x[b].T.reshape(8, 128, T).transpose(1, 0, 2)
            ).astype(ml_dtypes.bfloat16),
            "wqP": np.ascontiguousarray(
                wqkvT.reshape(8, 128, 3 * GD).transpose(1, 0, 2)
            ).astype(ml_dtypes.bfloat16),
            "woP": np.ascontiguousarray(
                qkvo_w[3][:, sl].T.reshape(2, 128, DIM).transpose(1, 0, 2)
            ).astype(ml_dtypes.bfloat16),
            "veP": np.ascontiguousarray(
                (ve[b][:, sl] * sa_lambdas[1]).reshape(NT, 128, GD)
                .transpose(1, 0, 2)
            ).astype(ml_dtypes.bfloat16),
            "lamP": lam0,
            "rcP": rcP,
            "rsP": rsP,
            "triP": tri01,
        }
        if has_gen:
            m["maskdiv"] = maskdiv
        in_maps.append(m)

    res = run_bass_kernel_spmd(nc, in_maps, core_ids=list(range(8)),
                               trace=TRACE)
    if TRACE:
        LAST_EXEC_NS = res.exec_time_ns

    out = np.zeros((B, T, DIM), np.float32)
    for c in range(8):
        out[c // G] += res.results[c]["outP"].astype(np.float32).T
    return out

